# revision 1
# baseline (speedup 1.0000x reference)
"""Trainium2 Bass kernel: causal multi-head attention (B=4, T=2048, D=1024, H=16).

Sharding: 2D (batch x head-group). Core c handles batch bg=c//2 and head
group hg=c%2 (8 heads = 512 projection dims). Each core:
  - projects q,k (transposed layout [hd, tok]) and v (natural [tok, hd],
    augmented with a ones column so PV also accumulates softmax row sums Z)
  - causal attention per head pair (S matmuls row-tiled on the PE via
    tile_position so both heads share one array pass)
  - partial output projection yT = wo_loc^T-contract o_loc^T  [1024, 2048]
Host gathers: y[bg] = yT[2*bg]^T + yT[2*bg+1]^T.

All matmul operands are fp16 (1 cycle/row on PE like bf16, FWL weight loads,
2x DVE copy modes, but 8x finer mantissa); PSUM accumulation is fp32.
Softmax 1/Z uses the single-op DVE reciprocal_approx_fast (~18 bits) and a
GpSimd partition broadcast; the normalize multiply is fused into the PSUM
evacuation of the attention output.
"""

import os
import numpy as np

import concourse.bass as bass
import concourse.bacc as bacc
import concourse.mybir as mybir
from concourse.tile import TileContext
from contextlib import ExitStack

# Problem constants (hardcoded per contract)
B, T, D, H = 4, 2048, 1024, 16
HD = D // H            # 64 head dim
P = 128                # partitions
KO = D // P            # 8 contraction subtiles for q/k/v projections
TQT = 512              # token tile width
NBLK = T // P          # 16 token blocks
HL = 8                 # heads per core
NPR = HL // 2          # head pairs per core (= contraction groups for wo)
DL = HL * HD           # 512 local projection dims
NCORES = 8
NCH = T // TQT         # 4 token chunks
SCALE = 1.0 / float(np.sqrt(np.float32(HD)))

F32 = mybir.dt.float32
MM_DT = getattr(mybir.dt, os.environ.get("BASS_MM_DT", "float16"))
DEBUG_DUMP = os.environ.get("BASS_DEBUG_DUMP") == "1"


def build_program():
    # All inputs are host-laid-out so every DMA slice is per-partition
    # contiguous (>=2KB lines for full DMA bandwidth).
    nc = bacc.Bacc("TRN2", target_bir_lowering=False, num_devices=NCORES)
    xt = nc.dram_tensor("xt", [P, NCH, KO, TQT], MM_DT, kind="ExternalInput")
    wq = nc.dram_tensor("wq", [P, KO, DL // P, P], MM_DT, kind="ExternalInput")
    wk = nc.dram_tensor("wk", [P, KO, DL // P, P], MM_DT, kind="ExternalInput")
    wv = nc.dram_tensor("wv", [P, KO, DL], MM_DT, kind="ExternalInput")
    wo = nc.dram_tensor("wo", [P, DL // P, D // P, P], MM_DT, kind="ExternalInput")
    cm = nc.dram_tensor("cmask", [P, P], MM_DT, kind="ExternalInput")     # tril keep
    y = nc.dram_tensor("y", [D, T], F32, kind="ExternalOutput")           # yT partial
    if DEBUG_DUMP:
        dbg_va = nc.dram_tensor("dbg_va", [P, NBLK, HL, HD + 1], MM_DT, kind="ExternalOutput")
        dbg_rz = nc.dram_tensor("dbg_rz", [1, 2, TQT], F32, kind="ExternalOutput")
        dbg_rzb = nc.dram_tensor("dbg_rzb", [P, 2, TQT], F32, kind="ExternalOutput")
        dbg_q = nc.dram_tensor("dbg_q", [P, NPR, TQT], MM_DT, kind="ExternalOutput")
        dbg_o = nc.dram_tensor("dbg_o", [P, NPR, TQT], MM_DT, kind="ExternalOutput")
        dbg_e = nc.dram_tensor("dbg_e", [P, 2, TQT], MM_DT, kind="ExternalOutput")

    y_r = y[:].rearrange("(mo p) t -> p mo t", p=P)

    Exp = mybir.ActivationFunctionType.Exp
    Copy = mybir.ActivationFunctionType.Copy
    Mult = mybir.AluOpType.mult

    with TileContext(nc) as tc, ExitStack() as ctx:
        const = ctx.enter_context(tc.tile_pool(name="const", bufs=1))
        work = ctx.enter_context(tc.tile_pool(name="work", bufs=1))
        e_pool = ctx.enter_context(tc.tile_pool(name="e", bufs=6))
        z_pool = ctx.enter_context(tc.tile_pool(name="z", bufs=4))
        zb_pool = ctx.enter_context(tc.tile_pool(name="zb", bufs=4))
        y_pool = ctx.enter_context(tc.tile_pool(name="yp", bufs=4))
        psS = ctx.enter_context(tc.tile_pool(name="psS", bufs=2, space="PSUM"))
        psO = ctx.enter_context(tc.tile_pool(name="psO", bufs=1, space="PSUM"))
        psA = ctx.enter_context(tc.tile_pool(name="psA", bufs=2, space="PSUM"))

        # --- constants into SBUF (ordered so chunk-0 projections can start
        # as soon as wq/wk + the first x chunk land) ---
        wq_sb = const.tile([P, KO, DL // P, P], MM_DT, tag="wq")
        wk_sb = const.tile([P, KO, DL // P, P], MM_DT, tag="wk")
        wv_sb = const.tile([P, KO, DL], MM_DT, tag="wv")
        wo_sb = const.tile([P, DL // P, D // P, P], MM_DT, tag="wo")
        cm_sb = const.tile([P, P], MM_DT, tag="cm")
        xt_sb = const.tile([P, NCH, KO, TQT], MM_DT, tag="xt")
        # cm first: the diagonal-mask multiply reads it via a to_broadcast AP
        # whose dependency is not reliably tracked — land it before anything
        # else can possibly consume it.
        nc.sync.dma_start(cm_sb, cm[:])
        nc.sync.dma_start(wq_sb[:, :, 0, :], wq[:, :, 0, :])
        nc.sync.dma_start(wk_sb[:, :, 0, :], wk[:, :, 0, :])
        # first x chunk arrives per ko-slice so the ko-0 matmuls of the first
        # projection group start ~3us before the full chunk lands
        for ko in range(KO):
            nc.sync.dma_start(xt_sb[:, 0, ko, :], xt[:, 0, ko, :])
        nc.sync.dma_start(wv_sb, wv[:])
        for mg in range(1, DL // P):
            nc.sync.dma_start(wq_sb[:, :, mg, :], wq[:, :, mg, :])
            nc.sync.dma_start(wk_sb[:, :, mg, :], wk[:, :, mg, :])
        for pc in range(1, NCH):
            nc.sync.dma_start(xt_sb[:, pc, :, :], xt[:, pc, :, :])
        nc.sync.dma_start(wo_sb, wo[:])

        # --- persistent work tiles ---
        qT = work.tile([P, NPR, T], MM_DT, tag="qT")     # [pair-dims, pr, tok]
        kT = work.tile([P, NPR, T], MM_DT, tag="kT")
        va = work.tile([P, NBLK, HL, HD + 1], MM_DT, tag="va")  # v natural + ones
        oT = work.tile([P, NPR, T], MM_DT, tag="oT")
        nc.vector.memset(va[:, :, :, HD : HD + 1], 1.0)

        def proj_chunk(pc):
            """q,k,v projections for token chunk pc (512 tokens)."""
            ts = slice(pc * TQT, (pc + 1) * TQT)
            for mg in range(DL // P):
                for w_sb, dst in ((wq_sb, qT), (wk_sb, kT)):
                    ps = psA.tile([P, TQT], F32, tag="psA")
                    for ko in range(KO):
                        nc.tensor.matmul(
                            ps,
                            w_sb[:, ko, mg, :],
                            xt_sb[:, pc, ko, :],
                            start=(ko == 0),
                            stop=(ko == KO - 1),
                        )
                    nc.vector.tensor_copy(dst[:, mg, ts], ps)
            for j in range(TQT // P):
                blk = pc * (TQT // P) + j
                ps = psA.tile([P, DL], F32, tag="psA")
                for ko in range(KO):
                    nc.tensor.matmul(
                        ps,
                        xt_sb[:, pc, ko, j * P : (j + 1) * P],
                        wv_sb[:, ko, :],
                        start=(ko == 0),
                        stop=(ko == KO - 1),
                    )
                nc.vector.tensor_copy(
                    va[:, blk, :, 0:HD],
                    ps[:].rearrange("p (h d) -> p h d", h=HL),
                )

        def attn_pair(qt, pr):
            """Attention for query chunk qt, head pair pr (local heads
            2pr, 2pr+1 at partitions 0-63 / 64-127 of group pr)."""
            tq0 = qt * TQT
            nblk = tq0 // P + TQT // P
            po = [
                psO.tile([HD + 1, TQT], F32, tag=f"po{h}", name=f"po{qt}_{pr}_{h}")
                for h in range(2)
            ]
            def kb_tail(kb, m, c0, ps2):
                """exp + causal mask + PV for one key block."""
                et2 = e_pool.tile([P, 2, TQT], MM_DT, tag="et")
                nc.scalar.activation(
                    et2[:, :, c0:TQT], ps2[:, :, c0:TQT], Exp, scale=SCALE
                )
                if m >= 0:
                    nc.vector.tensor_tensor(
                        et2[:, :, c0 : c0 + P],
                        et2[:, :, c0 : c0 + P],
                        cm_sb[:, None, :].to_broadcast((P, 2, P)),
                        Mult,
                    )
                if DEBUG_DUMP and qt == 0 and pr == 0 and kb == 0:
                    nc.sync.dma_start(dbg_e[:], et2)
                for h in range(2):
                    nc.tensor.matmul(
                        po[h][:, c0:TQT],
                        va[:, kb, 2 * pr + h, :],
                        et2[:, h, c0:TQT],
                        start=(kb == 0),
                        stop=(kb == nblk - 1),
                    )

            for kb in range(nblk):
                m = kb - tq0 // P
                c0 = P * m if m >= 0 else 0
                ps2 = psS.tile([P, 2, TQT], F32, tag="ps")
                for h in range(2):
                    hs = slice(h * HD, (h + 1) * HD)
                    nc.tensor.matmul(
                        ps2[:, h, c0:TQT],
                        kT[hs, pr, kb * P : (kb + 1) * P],
                        qT[hs, pr, tq0 + c0 : tq0 + TQT],
                        start=True,
                        stop=True,
                        tile_position=(h * HD, 0),
                    )
                kb_tail(kb, m, c0, ps2)
            # 1/Z (approx, ~18 bits) then broadcast across partitions and
            # normalize both heads while evacuating PSUM.
            zrow = z_pool.tile([1, 2, TQT], F32, tag="zrow")
            rz = z_pool.tile([1, 2, TQT], F32, tag="rz")
            rzb = zb_pool.tile([P, 2, TQT], F32, tag="rzb")
            for h in range(2):
                hs = slice(h * HD, (h + 1) * HD)
                nc.vector.tensor_copy(zrow[0:1, h, :], po[h][HD : HD + 1, :])
                nc.vector.reciprocal_approx_fast(rz[0:1, h, :], zrow[0:1, h, :])
                nc.gpsimd.partition_broadcast(rzb[:, h, :], rz[0:1, h, :])
                nc.vector.tensor_tensor(
                    oT[hs, pr, tq0 : tq0 + TQT],
                    po[h][0:HD, :],
                    rzb[hs, h, :],
                    Mult,
                )
            if DEBUG_DUMP and qt == 0 and pr == 0:
                nc.sync.dma_start(dbg_rz[:], rz)
                nc.sync.dma_start(dbg_rzb[:], rzb)
                nc.sync.dma_start(dbg_q[:], qT[:, :, 0:TQT])
                nc.sync.dma_start(dbg_o[:], oT[:, :, 0:TQT])
                nc.sync.dma_start(dbg_va[:], va)

        def outproj_piece(qt, mos):
            """Partial output projection yT[mos chunk, qt chunk]."""
            ts = slice(qt * TQT, (qt + 1) * TQT)
            for mo in mos:
                psy = psA.tile([P, TQT], F32, tag="psA")
                for ko in range(DL // P):
                    nc.tensor.matmul(
                        psy,
                        wo_sb[:, ko, mo, :],
                        oT[:, ko, ts],
                        start=(ko == 0),
                        stop=(ko == DL // P - 1),
                    )
                yt = y_pool.tile([P, TQT], F32, tag="yt")
                nc.vector.tensor_copy(yt, psy)
                nc.sync.dma_start(y_r[:, mo, ts], yt)

        def outproj_chunk(qt):
            outproj_piece(qt, range(D // P))

        # Emission order sets scheduler priority: attention (its ACT chain is
        # the latency-critical path) first, then projection/output-projection
        # fillers that keep the PE dense during exp round trips. The later qt
        # phases are ACT-bound, so output projections are deliberately pushed
        # into those windows.
        if os.environ.get("BASS_INTERLEAVE") == "1":
            proj_chunk(0)
            for qt in range(2):
                for pr in range(NPR):
                    attn_pair(qt, pr)
                    if pr == 0:
                        proj_chunk(qt + 1)
            for pr in range(NPR):
                attn_pair(2, pr)
                if pr == 0:
                    proj_chunk(3)
                attn_pair(3, pr)
                outproj_piece(0, range(pr * 2, pr * 2 + 2))
                outproj_piece(1, range(pr * 2, pr * 2 + 2))
                if pr == 3:
                    # qt2's oT is complete only now; these pieces fill the
                    # final (3,3) attention window
                    outproj_chunk(2)
        else:
            proj_chunk(0)
            for qt in range(NCH):
                for pr in range(NPR):
                    attn_pair(qt, pr)
                    if qt + 1 < NCH and pr == 0:
                        proj_chunk(qt + 1)
                    if qt == 3:
                        # all earlier chunks' output projections serve as PE
                        # filler inside the ACT-bound qt=3 window
                        outproj_piece(0, range(pr * 2, pr * 2 + 2))
                        outproj_piece(1, range(pr * 2, pr * 2 + 2))
                        if pr >= 1:
                            outproj_piece(2, range((pr - 1) * 3, min(8, pr * 3)))
        outproj_chunk(NCH - 1)

    nc.compile()
    return nc


def make_core_inputs(x, wq, wk, wv, wo):
    """Host-side sharding/layout prep. Returns list of 8 in_maps."""
    mdt = mybir.dt.np(MM_DT)
    x = np.asarray(x, dtype=np.float32)
    wq = np.asarray(wq, dtype=np.float32)
    wk = np.asarray(wk, dtype=np.float32)
    wv = np.asarray(wv, dtype=np.float32)
    wo = np.asarray(wo, dtype=np.float32)

    i = np.arange(P)[:, None]
    j = np.arange(P)[None, :]
    cmask = (i <= j).astype(mdt)  # keep-mask for diagonal 128x128 strips

    def pkq(a):  # [D, DL] -> [P, KO, MG, P]: a[ko*P+p, mg*P+q]
        return np.ascontiguousarray(
            a.reshape(KO, P, DL // P, P).transpose(1, 0, 2, 3)
        ).astype(mdt)

    in_maps = []
    for c in range(NCORES):
        bg, hg = c // 2, c % 2
        hs = slice(hg * DL, (hg + 1) * DL)
        xtb = x[bg].T  # [D, T]
        # [P, NCH, KO, TQT]: xtb[ko*P+p, pc*TQT+t]
        xtb = np.ascontiguousarray(
            xtb.reshape(KO, P, NCH, TQT).transpose(1, 2, 0, 3)
        ).astype(mdt)
        wvt = wv[hs, :].T  # [D, DL]
        wvt = np.ascontiguousarray(
            wvt.reshape(KO, P, DL).transpose(1, 0, 2)
        ).astype(mdt)
        wot = wo[:, hs].T  # [DL, D]: [ko*P+p, mo*P+q] -> [P, KO2, MO, P]
        wot = np.ascontiguousarray(
            wot.reshape(DL // P, P, D // P, P).transpose(1, 0, 2, 3)
        ).astype(mdt)
        in_maps.append(
            {
                "xt": xtb,
                "wq": pkq(wq[hs, :].T),
                "wk": pkq(wk[hs, :].T),
                "wv": wvt,
                "wo": wot,
                "cmask": cmask,
            }
        )
    return in_maps


_CACHE = {}


def run(in_maps, **kwargs):
    from concourse.bass_utils import run_bass_kernel_spmd

    if "nc" not in _CACHE:
        _CACHE["nc"] = build_program()
    nc = _CACHE["nc"]
    res = run_bass_kernel_spmd(nc, in_maps, core_ids=list(range(NCORES)), **kwargs)
    return res


def gather(results):
    y = np.empty((B, T, D), dtype=np.float32)
    for bg in range(B):
        yT = results[2 * bg]["y"] + results[2 * bg + 1]["y"]
        y[bg] = yT.T
    return y


def kernel(x, wq, wk, wv, wo):
    in_maps = make_core_inputs(x, wq, wk, wv, wo)
    res = run(in_maps)
    return gather([r for r in res.results])

